# revision 1
# baseline (speedup 1.0000x reference)
"""DeepseekV3 MoE gate kernel for 8 TRN2 NeuronCores.

Strategy (per sharding hint): shard tokens 8192 -> 8 x 1024, replicate gate
weight/bias. Per core:
  - GEMM logits^T[e,t] = W @ x^T via fp16 hi/lo 3-pass (exact to ~3e-7):
    x tiles are PE-transposed (fp32, exact), the PSUM->SBUF copy splits into
    fp16 hi (ACT) + lo (DVE). Weights are host-split/transposed (static prep).
  - sigmoid (ACT) + bias add (DVE, per-partition) in [e,t] layout, then PE
    output-transpose to [t,e] for the per-token top-k.
  - top-k: grouped reduce_max + match_replace for per-group top-2 sums,
    vector.max for group/expert thresholds, is_ge masks, normalize.
"""
import sys

for _p in ("/opt/trn_rl_repo", "/opt/pypackages"):
    if _p not in sys.path:
        sys.path.append(_p)

import numpy as np
import concourse.bacc as bacc
import concourse.mybir as mybir
import concourse.tile as tile
from concourse import bass_utils
from concourse.masks import make_identity

F32 = mybir.dt.float32
F16 = mybir.dt.float16
AF = mybir.ActivationFunctionType
OP = mybir.AluOpType
AX = mybir.AxisListType

TOKENS, HIDDEN, E = 8192, 7168, 256
N_CORES = 8
T = TOKENS // N_CORES          # 1024 tokens per core
KT = HIDDEN // 128             # 56 k-tiles
KS = 4                         # k-tiles per x-slice load ([128, 512])
NSLICE = KT // KS              # 14 h-slices
CHUNK = 512                    # tokens per pipeline chunk
NCH = T // CHUNK               # 2 chunks
TB = CHUNK // 128              # 4 token blocks per chunk
ROUTED_SCALING = 2.5

_CACHE = {}


def _build():
    nc = bacc.Bacc("TRN2", target_bir_lowering=False, debug=False)
    x_d = nc.dram_tensor("x", [T, HIDDEN], F32, kind="ExternalInput").ap()
    wht_d = nc.dram_tensor("wht", [128, KT, E], F16, kind="ExternalInput").ap()
    wlt_d = nc.dram_tensor("wlt", [128, KT, E], F16, kind="ExternalInput").ap()
    bias_d = nc.dram_tensor("bias_pe", [128, 2], F32, kind="ExternalInput").ap()
    out_d = nc.dram_tensor("out", [T, E], F32, kind="ExternalOutput").ap()

    with tile.TileContext(nc) as tc:
        with tc.tile_pool(name="const", bufs=1) as const, \
             tc.tile_pool(name="xring", bufs=44) as xring, \
             tc.tile_pool(name="xt16", bufs=4) as xt16, \
             tc.tile_pool(name="etile", bufs=2) as etile, \
             tc.tile_pool(name="tk", bufs=3) as tkp, \
             tc.tile_pool(name="pacc", bufs=2, space="PSUM") as pacc, \
             tc.tile_pool(name="pxt", bufs=2, space="PSUM") as pxt, \
             tc.tile_pool(name="pot", bufs=2, space="PSUM") as pot:

            wht_t = const.tile([128, KT, E], F16, name="wht_t")
            nc.sync.dma_start(wht_t, wht_d)
            wlt_t = const.tile([128, KT, E], F16, name="wlt_t")
            nc.sync.dma_start(wlt_t, wlt_d)
            bias_t = const.tile([128, 2], F32, name="bias_t")
            nc.sync.dma_start(bias_t, bias_d)
            ident = const.tile([128, 128], F32, name="ident")
            make_identity(nc, ident)

            for c in range(NCH):
                accs = [pacc.tile([128, CHUNK], F32, name=f"acc{e}_{c}", tag=f"acc{e}")
                        for e in range(2)]
                xnat = {}
                for k in range(KT):
                    ks, ko = divmod(k, KS)
                    if ko == 0:
                        for t4 in (0, 1, 2, 3):
                            xn = xring.tile([128, 512], F32, name=f"xn_{c}_{t4}_{ks}",
                                            tag="xn")
                            r0 = c * CHUNK + t4 * 128
                            nc.sync.dma_start(
                                xn, x_d[r0:r0 + 128, ks * 512:(ks + 1) * 512])
                            xnat[t4] = xn
                    xt_ps = pxt.tile([128, CHUNK], F32, name=f"xt_{c}_{k}", tag="xt")
                    for t4 in (0, 1, 2, 3):
                        nc.tensor.transpose(
                            xt_ps[:, t4 * 128:(t4 + 1) * 128],
                            xnat[t4][:, ko * 128:(ko + 1) * 128], ident)
                    hi = xt16.tile([128, CHUNK], F16, name=f"hi_{c}_{k}", tag="hi")
                    nc.scalar.copy(hi, xt_ps)
                    lo = xt16.tile([128, CHUNK], F16, name=f"lo_{c}_{k}", tag="lo")
                    nc.vector.tensor_tensor(lo, xt_ps, hi, op=OP.subtract)
                    first, last = (k == 0), (k == KT - 1)
                    for e in range(2):
                        esl = slice(e * 128, (e + 1) * 128)
                        nc.tensor.matmul(accs[e], wht_t[:, k, esl], hi,
                                         start=first, stop=False)
                        nc.tensor.matmul(accs[e], wlt_t[:, k, esl], hi,
                                         start=False, stop=False)
                        nc.tensor.matmul(accs[e], wht_t[:, k, esl], lo,
                                         start=False, stop=last)

                # [e, t] epilogue: sigmoid + bias
                sc_et = etile.tile([128, 2, CHUNK], F32, name=f"sc_et_{c}", tag="sc")
                swb_et = etile.tile([128, 2, CHUNK], F32, name=f"swb_et_{c}", tag="swb")
                for e in range(2):
                    nc.scalar.activation(sc_et[:, e], accs[e], AF.Sigmoid)
                    nc.vector.tensor_scalar_add(swb_et[:, e], sc_et[:, e],
                                                bias_t[:, e:e + 1])

                # transpose to [t, e] and run top-k per 128-token block
                for t4 in range(TB):
                    tsl = slice(t4 * 128, (t4 + 1) * 128)
                    ot_ps = pot.tile([128, 512], F32, name=f"ot_{c}_{t4}", tag="ot")
                    for e in range(2):
                        nc.tensor.transpose(ot_ps[:, e * 128:(e + 1) * 128],
                                            swb_et[:, e, tsl], ident)
                        nc.tensor.transpose(ot_ps[:, 256 + e * 128:256 + (e + 1) * 128],
                                            sc_et[:, e, tsl], ident)
                    tk = tkp.tile([128, 512], F32, name=f"tk_{c}_{t4}", tag="tk")
                    nc.scalar.copy(tk, ot_ps)
                    swb = tk[:, 0:256]
                    scores = tk[:, 256:512]
                    swb_g = swb.rearrange("p (g s) -> p g s", s=32)

                    m1 = tkp.tile([128, 8], F32, name=f"m1_{c}_{t4}", tag="m1")
                    nc.vector.tensor_reduce(m1, swb_g, axis=AX.X, op=OP.max)
                    swb2 = tkp.tile([128, 256], F32, name=f"swb2_{c}_{t4}", tag="swb2")
                    nc.vector.match_replace(out=swb2, in_to_replace=m1,
                                            in_values=swb, imm_value=-1e30)
                    gsum = tkp.tile([128, 8], F32, name=f"gsum_{c}_{t4}", tag="gsum")
                    nc.vector.tensor_reduce(gsum, swb2.rearrange("p (g s) -> p g s", s=32),
                                            axis=AX.X, op=OP.max)
                    nc.vector.tensor_tensor(gsum, gsum, m1, op=OP.add)
                    g8 = tkp.tile([128, 8], F32, name=f"g8_{c}_{t4}", tag="g8")
                    nc.vector.max(out=g8, in_=gsum)
                    gmask = tkp.tile([128, 8], F32, name=f"gmask_{c}_{t4}", tag="gmask")
                    nc.vector.tensor_scalar(gmask, gsum, g8[:, 3:4], None, op0=OP.is_ge)
                    swbm = tkp.tile([128, 256], F32, name=f"swbm_{c}_{t4}", tag="swbm")
                    nc.vector.tensor_tensor(
                        swbm.rearrange("p (g s) -> p g s", s=32), swb_g,
                        gmask[:, :, None].to_broadcast([128, 8, 32]), op=OP.mult)
                    top8 = tkp.tile([128, 8], F32, name=f"top8_{c}_{t4}", tag="top8")
                    nc.vector.max(out=top8, in_=swbm)
                    emask = tkp.tile([128, 256], F32, name=f"emask_{c}_{t4}", tag="emask")
                    nc.vector.tensor_scalar(emask, swbm, top8[:, 7:8], None, op0=OP.is_ge)
                    sel = tkp.tile([128, 256], F32, name=f"sel_{c}_{t4}", tag="sel")
                    nc.vector.tensor_tensor(sel, scores, emask, op=OP.mult)
                    ssum = tkp.tile([128, 1], F32, name=f"ssum_{c}_{t4}", tag="ssum")
                    nc.vector.tensor_reduce(ssum, sel, axis=AX.X, op=OP.add)
                    inv = tkp.tile([128, 1], F32, name=f"inv_{c}_{t4}", tag="inv")
                    nc.vector.reciprocal(inv, ssum)
                    ow = tkp.tile([128, 256], F32, name=f"ow_{c}_{t4}", tag="ow")
                    nc.vector.tensor_scalar(ow, sel, inv, ROUTED_SCALING,
                                            op0=OP.mult, op1=OP.mult)
                    r0 = c * CHUNK + t4 * 128
                    nc.sync.dma_start(out_d[r0:r0 + 128, :], ow)

    nc.compile()
    return nc


def _prep_weights(weight, bias):
    w = np.asarray(weight, np.float32)
    wh = w.astype(np.float16)
    wl = (w - wh.astype(np.float32)).astype(np.float16)

    def lay(a):
        # [E, H] -> W^T tiles [128part(h), KT, E], contiguous
        return np.ascontiguousarray(
            a.T.reshape(KT, 128, E).transpose(1, 0, 2))

    bias_pe = np.ascontiguousarray(
        np.asarray(bias, np.float32).reshape(2, 128).T)
    return lay(wh), lay(wl), bias_pe


def kernel(hidden_states, weight, e_score_correction_bias):
    x = np.ascontiguousarray(np.asarray(hidden_states, np.float32))
    wht, wlt, bias_pe = _prep_weights(weight, e_score_correction_bias)
    if "nc" not in _CACHE:
        _CACHE["nc"] = _build()
    nc = _CACHE["nc"]
    in_maps = [
        {"x": x[i * T:(i + 1) * T], "wht": wht, "wlt": wlt, "bias_pe": bias_pe}
        for i in range(N_CORES)
    ]
    res = bass_utils.run_bass_kernel_spmd(nc, in_maps, core_ids=list(range(N_CORES)))
    return np.concatenate([res.results[i]["out"] for i in range(N_CORES)], axis=0)


if __name__ == "__main__":
    rng = np.random.default_rng(0)
    hs = rng.standard_normal((TOKENS, HIDDEN)).astype(np.float32)
    w = (rng.standard_normal((E, HIDDEN)) * 0.02).astype(np.float32)
    b = (rng.standard_normal(E) * 0.1).astype(np.float32)
    out = kernel(hs, w, b)
    print(out.shape, out.dtype, np.isfinite(out).all())
